# revision 35
# baseline (speedup 1.0000x reference)
"""Trainium2 Bass kernel for nn_Attn_90623809945974.

out[b, 0, :] = softmax_l( hidden[0,b,:] . (W @ enc[l,b,:] + bias) )
             = softmax_l( (W^T h_b) . enc[l,b,:] )   (bias const per b -> cancels)

Sharding: data-parallel over batch (B=64 -> 8 per core); W replicated.

v7 design (PE-matmul energies over fp16+fp8 split-precision enc):
  - The serial 360 GB/s DMA device is the roofline; v6 shipped enc fp32
    (64 MiB/core, 186.4us -> 198us total). v7 ships enc as fp16 hi + fp8
    (e4m3) lo planes (48 MiB/core, 139.8us): enc = hi + lo/128, effective
    precision ~2^-15 (measured end-to-end max rel err 4.3e-3 vs 2e-2 gate).
  - Host pre-transposes both planes so the contraction dim h lands on SBUF
    partitions; energies come from PE matmuls (fp16xfp16 / fp8xfp8 products
    are exact in fp32 PSUM; verified on HW):
      ps_hi[128l, 8b] += hiT(j,b)[128h,128l]^T @ vmh/vml_masked(j)[128h,8]
      ps_lo[128l, 8b] += loT(j,b)^T @ vm8_masked(j)
    96 matmuls/tile at ~3ns engine each; PE replaces the DVE stt pipeline
    (mode-less at 594ns/(b,tile), which would cap any byte-shrink at 152us).
  - W also ships as fp16 + fp8(residual*2^13) planes (one strided
    rearrange-DMA each; many small DMAs would starve the stream via their
    625ns HWDGE cost). v = W^T h_b computed on device from Wh.(hh+hl) +
    2^-11 * Wl.(h/4) with hid split on device; v then splits into
    vmh=fp16(v), vml=fp16(v-vmh), vm8=fp8(v/2); per-tile evac:
    eacc = ps_hi + 2^-6 * ps_lo (ACT copy-scale + DVE add, [128,8]).
  - Last l-group ships in a separate l-tile-major layout, DMA'd per
    (l-tile, plane) with lo before hi and the very last tile's hi plane as
    four per-j slices; matmul emission is j-outer, so after the final DMA
    sem only one j's matmuls remain. S over chunks 0..9 is pre-reduced
    while the last tile streams, leaving one add + reciprocal at the end.
  - Downstream softmax machinery carried over from v6: 7 big 512-l chunks
    shifted by own max, tail chunks shifted by m_prev with fused PSUM-evac
    exp+accum, premultiply of big chunks by exp(m_c-m_prev) on idle DVE,
    three-engine 1/S rescale, bf16 output (host upcasts).
"""
import numpy as np
import ml_dtypes

import concourse.bass as bass
import concourse.bacc as bacc
import concourse.mybir as mybir
from concourse import tile, masks
from concourse.bass_utils import run_bass_kernel_spmd

L = 4096
B = 64
H = 512
NCORES = 8
BL = B // NCORES   # 8
LT = 128           # l-tile rows
LC = 512           # l-group == big softmax chunk
NG = L // LC       # 8
TPG = LC // LT     # 4
NJ = H // 128      # 4
F32 = mybir.dt.float32
F16 = mybir.dt.float16
F8 = mybir.dt.float8e4
BF16 = mybir.dt.bfloat16
A = mybir.AluOpType
LO_SCALE = 128.0                            # lo = fp8(r * 128)
V8_SCALE = 0.5                              # vm8 = fp8(v * 0.5)
LO_COMBINE = 1.0 / (LO_SCALE * V8_SCALE)    # 2^-6

_cache = {}


def _build(num_devices=NCORES, do_compile=True):
    NT = L // LT                     # 32 l-tiles
    NCH = NG                         # 8
    NBIG = NCH - 1                   # 7 big chunks
    GW = NJ * BL * LC                # group tile width (16384)
    nc = bacc.Bacc("TRN2", target_bir_lowering=False, debug=False, num_devices=num_devices)
    hi_d = nc.dram_tensor("hi", [NJ * NBIG * 128, BL * LC], F16, kind="ExternalInput").ap()
    lo_d = nc.dram_tensor("lo", [NJ * NBIG * 128, BL * LC], F8, kind="ExternalInput").ap()
    hit_d = nc.dram_tensor("hit", [NJ * TPG * 128, BL * LT], F16, kind="ExternalInput").ap()
    lot_d = nc.dram_tensor("lot", [NJ * TPG * 128, BL * LT], F8, kind="ExternalInput").ap()
    hid_d = nc.dram_tensor("hid", [BL, H], F32, kind="ExternalInput").ap()
    wh_d = nc.dram_tensor("wh", [H, H], F16, kind="ExternalInput").ap()
    wl_d = nc.dram_tensor("wl", [H, H], F8, kind="ExternalInput").ap()
    out_d = nc.dram_tensor("out", [BL, L], BF16, kind="ExternalOutput").ap()

    with tile.TileContext(nc) as tc:
        with (
            tc.tile_pool(name="const", bufs=1) as constp,
            tc.tile_pool(name="keep", bufs=1) as keepp,
            tc.tile_pool(name="hig", bufs=2) as higp,
            tc.tile_pool(name="log", bufs=2) as logp,
            tc.tile_pool(name="eacc", bufs=4) as eaccp,
            tc.tile_pool(name="eaccl", bufs=4) as eacclp,
            tc.tile_pool(name="ehps", bufs=2, space="PSUM") as ehps,
            tc.tile_pool(name="elps", bufs=2, space="PSUM") as elps,
            tc.tile_pool(name="tpsum", bufs=2, space="PSUM") as tpsum,
            tc.tile_pool(name="smt", bufs=2) as smtp,
            tc.tile_pool(name="post", bufs=1) as postp,
        ):
            # softmax chunks: 7x512 then 4x128 (short tail chunks)
            SCH = [(i * LC, LC) for i in range(NBIG)]
            SCH += [(NBIG * LC + i * LT, LT) for i in range(TPG)]
            NSC = len(SCH)
            tile_ends_chunk = {}
            for ci, (s0, sz) in enumerate(SCH):
                tile_ends_chunk[(s0 + sz) // LT - 1] = ci

            ident = constp.tile([128, 128], F32, tag="ident")
            esm = keepp.tile([BL, L], F32, tag="esm")
            mstore = keepp.tile([BL, NBIG], F32, tag="mstore")
            sstore = keepp.tile([BL, NSC], F32, tag="sstore")
            negmp = keepp.tile([BL, 1], F32, tag="negmp")    # -max(m_0..m_6)
            esmb = keepp.tile([BL, L], BF16, tag="esmb")
            ffac = keepp.tile([BL, NSC], F32, tag="ffac")
            # masked v tiles: block (b,j) = cols [(b*NJ+j)*8, +8), column b live
            vmh = keepp.tile([128, BL * NJ * 8], F16, tag="vmh")
            vml = keepp.tile([128, BL * NJ * 8], F16, tag="vml")
            vm8 = keepp.tile([128, BL * NJ * 8], F8, tag="vm8")
            ssum = postp.tile([BL, 1], F32, tag="ssum")
            ssum_p = postp.tile([BL, 1], F32, tag="ssum_p")
            ssum2 = postp.tile([BL, 1], F32, tag="ssum2")
            rsum = postp.tile([BL, 1], F32, tag="rsum")
            wfac = postp.tile([BL, NSC], F32, tag="wfac")

            # ---------- DMA issue: per-group j-planes; last group sliced ----
            hi_tiles = {}
            lo_tiles = {}

            def issue_group_dma(g, part=None):
                """part=None: whole group. For g==NG-1 use part=t (issue per
                l-tile slice, called TPG times)."""
                if g < NBIG:
                    ht = higp.tile([128, GW], F16, tag="hi_g", name="hi_gm")
                    lt_ = logp.tile([128, GW], F8, tag="lo_g", name="lo_gm")
                    for j in range(NJ):
                        nc.sync.dma_start(
                            ht[:, j * BL * LC:(j + 1) * BL * LC],
                            hi_d[(j * NBIG + g) * 128:(j * NBIG + g + 1) * 128, :])
                        nc.sync.dma_start(
                            lt_[:, j * BL * LC:(j + 1) * BL * LC],
                            lo_d[(j * NBIG + g) * 128:(j * NBIG + g + 1) * 128, :])
                    hi_tiles[g] = ht
                    lo_tiles[g] = lt_
                else:
                    # tail group, t-major layout: one DMA per (t, plane)
                    t = part
                    if t == 0:
                        hi_tiles[g] = higp.tile([128, GW], F16, tag="hi_g", name="hi_gt")
                        lo_tiles[g] = logp.tile([128, GW], F8, tag="lo_g", name="lo_gt")
                    ht, lt_ = hi_tiles[g], lo_tiles[g]
                    TW = NJ * BL * LT       # 4096 cols per l-tile
                    # lo first: its matmuls run during the hi transfer, so
                    # only hi-matmuls sit behind the final DMA sem
                    nc.sync.dma_start(
                        lt_[:, t * TW:(t + 1) * TW],
                        lot_d[t * NJ * 128:(t + 1) * NJ * 128, :].rearrange(
                            "(j p) w -> p j w", j=NJ))
                    if t == TPG - 1:
                        # very last tile: per-j hi slices + j-outer matmul
                        # emission leave only one j's matmuls post-sem
                        for j in range(NJ):
                            nc.sync.dma_start(
                                ht[:, t * TW + j * BL * LT:
                                   t * TW + (j + 1) * BL * LT],
                                hit_d[(t * NJ + j) * 128:
                                      (t * NJ + j + 1) * 128, :])
                    else:
                        nc.sync.dma_start(
                            ht[:, t * TW:(t + 1) * TW],
                            hit_d[t * NJ * 128:(t + 1) * NJ * 128, :].rearrange(
                                "(j p) w -> p j w", j=NJ))

            def lhs_slice(g, tg, j, b, tiles):
                if g < NBIG:
                    off = j * (BL * LC) + b * LC + tg * LT
                else:
                    off = (tg * NJ + j) * (BL * LT) + b * LT
                return tiles[g][:, off:off + LT]

            # first hi DMA of group 0 owns the DMA engines early
            g0h = higp.tile([128, GW], F16, tag="hi_g")
            g0l = logp.tile([128, GW], F8, tag="lo_g")
            nc.sync.dma_start(g0h[:, 0:BL * LC], hi_d[0:128, :])

            with (
                tc.tile_pool(name="pre", bufs=1) as prep,
                tc.tile_pool(name="prepsum", bufs=2, space="PSUM") as prepsum,
            ):
                # W (fp16 + fp8 residual planes) + hid after the first enc DMA
                hid_sb = prep.tile([BL, H], F32, tag="hid_sb")
                nc.sync.dma_start(hid_sb[:], hid_d[:])
                wh_sb = prep.tile([128, 4 * H], F16, tag="wh_sb")
                wl_sb = prep.tile([128, 4 * H], F8, tag="wl_sb")
                nc.sync.dma_start(
                    wh_sb[:], wh_d[:].rearrange("(j p) w -> p j w", j=4))
                nc.sync.dma_start(
                    wl_sb[:], wl_d[:].rearrange("(j p) w -> p j w", j=4))
                # rest of group 0
                for j in range(NJ):
                    if j > 0:
                        nc.sync.dma_start(
                            g0h[:, j * BL * LC:(j + 1) * BL * LC],
                            hi_d[(j * NBIG) * 128:(j * NBIG + 1) * 128, :])
                    nc.sync.dma_start(
                        g0l[:, j * BL * LC:(j + 1) * BL * LC],
                        lo_d[(j * NBIG) * 128:(j * NBIG + 1) * 128, :])
                hi_tiles[0] = g0h
                lo_tiles[0] = g0l
                issue_group_dma(1)

                masks.make_identity(nc, ident[:])

                # h8t[p, j*8+b] = hid[b, j*128+p]; split to fp16 hi/lo + fp8
                h8t = prep.tile([128, 4 * BL], F32, tag="h8t")
                for j in range(4):
                    ps = prepsum.tile([128, BL], F32, tag="pre_ps")
                    nc.tensor.transpose(ps[:], hid_sb[:, j * 128:(j + 1) * 128], ident[:BL, :BL])
                    nc.vector.tensor_copy(h8t[:, j * BL:(j + 1) * BL], ps[:])
                h8h = prep.tile([128, 4 * BL], F16, tag="h8h")
                nc.vector.tensor_copy(h8h[:], h8t[:])
                h8h32 = prep.tile([128, 4 * BL], F32, tag="h8h32")
                nc.vector.tensor_copy(h8h32[:], h8h[:])
                h8l32 = prep.tile([128, 4 * BL], F32, tag="h8l32")
                nc.vector.tensor_tensor(out=h8l32[:], in0=h8t[:], in1=h8h32[:], op=A.subtract)
                h8l = prep.tile([128, 4 * BL], F16, tag="h8l")
                nc.vector.tensor_copy(h8l[:], h8l32[:])
                h8q32 = prep.tile([128, 4 * BL], F32, tag="h8q32")
                nc.vector.tensor_scalar_mul(h8q32[:], h8t[:], 0.25)
                h8q = prep.tile([128, 4 * BL], F8, tag="h8q")
                nc.vector.tensor_copy(h8q[:], h8q32[:])

                # v8f[o, j*8+b] = v[b, j*128+o],  v_b = W^T h_b
                #   = Wh.(hh + hl) + (Wl/2^13).(hq*4)
                v8f = prep.tile([128, 4 * BL], F32, tag="v8f")
                for j in range(4):
                    psv = prepsum.tile([128, BL], F32, tag="pre_ps")
                    for i in range(4):
                        nc.tensor.matmul(
                            psv[:],
                            wh_sb[:, i * H + j * 128: i * H + (j + 1) * 128],
                            h8h[:, i * BL:(i + 1) * BL],
                            start=(i == 0), stop=False,
                        )
                        nc.tensor.matmul(
                            psv[:],
                            wh_sb[:, i * H + j * 128: i * H + (j + 1) * 128],
                            h8l[:, i * BL:(i + 1) * BL],
                            start=False, stop=(i == 3),
                        )
                    psl = prepsum.tile([128, BL], F32, tag="pre_ps")
                    for i in range(4):
                        nc.tensor.matmul(
                            psl[:],
                            wl_sb[:, i * H + j * 128: i * H + (j + 1) * 128],
                            h8q[:, i * BL:(i + 1) * BL],
                            start=(i == 0), stop=(i == 3),
                        )
                    vtmp = prep.tile([128, BL], F32, tag="vtmp", name=f"vtmp{j}")
                    nc.scalar.activation(
                        out=vtmp[:], in_=psl[:],
                        func=mybir.ActivationFunctionType.Copy,
                        scale=float(2.0 ** -13 * 4.0))
                    nc.vector.tensor_tensor(
                        out=v8f[:, j * BL:(j + 1) * BL], in0=psv[:], in1=vtmp[:],
                        op=A.add)

                # split v: vh16 = fp16(v), vl16 = fp16(v - vh16), v88 = fp8(v/2)
                vh16 = prep.tile([128, 4 * BL], F16, tag="vh16")
                nc.vector.tensor_copy(vh16[:], v8f[:])
                vh32 = prep.tile([128, 4 * BL], F32, tag="vh32")
                nc.vector.tensor_copy(vh32[:], vh16[:])
                vl32 = prep.tile([128, 4 * BL], F32, tag="vl32")
                nc.vector.tensor_tensor(out=vl32[:], in0=v8f[:], in1=vh32[:], op=A.subtract)
                vl16 = prep.tile([128, 4 * BL], F16, tag="vl16")
                nc.vector.tensor_copy(vl16[:], vl32[:])
                v8h = prep.tile([128, 4 * BL], F32, tag="v8h")
                nc.vector.tensor_scalar_mul(v8h[:], v8f[:], V8_SCALE)
                v88 = prep.tile([128, 4 * BL], F8, tag="v88")
                nc.vector.tensor_copy(v88[:], v8h[:])

                # masked tiles: block (b,j) col b <- v*[:, j*8+b]
                nc.vector.memset(vmh[:], 0.0)
                nc.vector.memset(vml[:], 0.0)
                nc.vector.memset(vm8[:], 0.0)
                for b in range(BL):
                    for j in range(NJ):
                        blk = (b * NJ + j) * 8
                        nc.vector.tensor_copy(
                            vmh[:, blk + b: blk + b + 1],
                            vh16[:, j * BL + b: j * BL + b + 1])
                        nc.vector.tensor_copy(
                            vml[:, blk + b: blk + b + 1],
                            vl16[:, j * BL + b: j * BL + b + 1])
                        nc.vector.tensor_copy(
                            vm8[:, blk + b: blk + b + 1],
                            v88[:, j * BL + b: j * BL + b + 1])

            # ---------- main tile loop ----------
            issued = [2]     # groups 0,1 already issued
            premult = []     # big chunks awaiting their exp(m_c - m_prev) factor

            def issue_ahead(t):
                g = t // TPG
                # keep 2 groups in flight beyond the one being consumed
                while issued[0] < min(g + 3, NG):
                    gg = issued[0]
                    if gg < NBIG:
                        issue_group_dma(gg)
                    else:
                        for tt in range(TPG):
                            issue_group_dma(gg, part=tt)
                    issued[0] += 1

            for t in range(NT):
                issue_ahead(t)
                g, tg = divmod(t, TPG)
                # fold the combine factor into big slices on idle DVE slack
                for _ in range(2):
                    if t >= NT - 4 and premult:
                        cb = premult.pop(0)
                        nc.vector.tensor_scalar_mul(
                            esmb[:, cb * LC:(cb + 1) * LC],
                            esm[:, cb * LC:(cb + 1) * LC],
                            ffac[:, cb:cb + 1])

                if t == NT - 1:
                    # S over chunks 0..NSC-2 while the last tile streams in;
                    # the final S is then one small add after the last exp
                    nc.vector.scalar_tensor_tensor(
                        out=wfac[:, 0:NSC - 1], in0=ffac[:, 0:NSC - 1],
                        scalar=1.0, in1=sstore[:, 0:NSC - 1],
                        op0=A.mult, op1=A.mult, accum_out=ssum_p[:])

                # lo-plane matmuls first: in the tail group lo is DMA'd ahead
                # of hi, so only the hi-matmuls sit behind the final DMA sem
                ps_h = ehps.tile([128, BL], F32, tag="ps_h")
                ps_l = elps.tile([128, BL], F32, tag="ps_l")
                kl = 0
                for j in range(NJ):
                    for b in range(BL):
                        blk = (b * NJ + j) * 8
                        nc.tensor.matmul(
                            ps_l[:], lhs_slice(g, tg, j, b, lo_tiles),
                            vm8[:, blk:blk + 8],
                            start=(kl == 0), stop=(kl == BL * NJ - 1))
                        kl += 1
                n_h = BL * NJ * 2
                k = 0
                for j in range(NJ):
                    for b in range(BL):
                        blk = (b * NJ + j) * 8
                        nc.tensor.matmul(
                            ps_h[:], lhs_slice(g, tg, j, b, hi_tiles),
                            vmh[:, blk:blk + 8],
                            start=(k == 0), stop=(k == n_h - 1))
                        k += 1
                        nc.tensor.matmul(
                            ps_h[:], lhs_slice(g, tg, j, b, hi_tiles),
                            vml[:, blk:blk + 8],
                            start=False, stop=(k == n_h - 1))
                        k += 1

                # evac: eacc = ps_h + 2^-6 * ps_l (HW allows only one PSUM
                # read per DVE op, so ACT scales ps_l out first)
                eacc_l = eacclp.tile([128, BL], F32, tag="eacc_l")
                nc.scalar.activation(
                    out=eacc_l[:], in_=ps_l[:],
                    func=mybir.ActivationFunctionType.Copy, scale=LO_COMBINE)
                eacc = eaccp.tile([128, BL], F32, tag="eacc")
                nc.vector.tensor_tensor(out=eacc[:], in0=ps_h[:], in1=eacc_l[:], op=A.add)

                pe_t = tpsum.tile([BL, LT], F32, tag="pe_t")
                nc.tensor.transpose(pe_t[:], eacc[:], ident[:])

                base = t * LT
                ci = tile_ends_chunk.get(t)
                if ci is not None and ci >= NBIG:
                    # tail tile-chunk: fused PSUM-evac + exp(x - m_prev),
                    # accumulating s_c
                    nc.scalar.activation(
                        out=esm[:, base:base + LT], in_=pe_t[:],
                        func=mybir.ActivationFunctionType.Exp,
                        bias=negmp[:], scale=1.0, accum_out=sstore[:, ci:ci + 1])
                    continue
                nc.scalar.copy(esm[:, base:base + LT], pe_t[:])

                if ci is None:
                    continue
                sl = esm[:, SCH[ci][0]:SCH[ci][0] + SCH[ci][1]]
                # big chunk: shift by own max
                mx_c = smtp.tile([BL, 1], F32, tag=f"mx_{ci % 2}")
                nc.vector.tensor_reduce(
                    out=mx_c[:], in_=sl, axis=mybir.AxisListType.X, op=A.max)
                nc.vector.tensor_copy(mstore[:, ci:ci + 1], mx_c[:])
                negm = smtp.tile([BL, 1], F32, tag=f"ng_{ci % 2}")
                nc.vector.tensor_scalar_mul(negm[:], mx_c[:], -1.0)
                nc.scalar.activation(
                    out=sl, in_=sl,
                    func=mybir.ActivationFunctionType.Exp,
                    bias=negm[:], scale=1.0, accum_out=sstore[:, ci:ci + 1])
                if ci == NBIG - 1:
                    # m_prev = max over big-chunk maxes; combine factors
                    nc.vector.tensor_reduce(
                        out=negmp[:], in_=mstore[:],
                        axis=mybir.AxisListType.X, op=A.max)
                    nc.vector.tensor_scalar_mul(negmp[:], negmp[:], -1.0)
                    nc.vector.memset(ffac[:, NBIG:], 1.0)
                    nc.scalar.activation(
                        out=ffac[:, 0:NBIG], in_=mstore[:],
                        func=mybir.ActivationFunctionType.Exp,
                        bias=negmp[:], scale=1.0)
                    premult.extend(range(NBIG))

            # ---------- epilogue: S = ssum_p + s_last; x 1/S; store ---------
            nc.vector.tensor_tensor(
                out=ssum[:], in0=ssum_p[:], in1=sstore[:, NSC - 1:NSC], op=A.add)
            nc.vector.tensor_copy(ssum2[:], ssum[:])
            nc.vector.reciprocal(rsum[:], ssum[:])

            # x 1/S in three engine regions, casting to bf16 on write
            D0, D1 = 2848, 3584
            nc.vector.tensor_scalar_mul(esmb[:, 0:D0], esmb[:, 0:D0], rsum[:])
            nc.scalar.activation(
                out=esmb[:, D0:D1], in_=esmb[:, D0:D1],
                func=mybir.ActivationFunctionType.Copy, scale=rsum[:])
            nc.gpsimd.normalize_recip(esmb[:, D1:], esm[:, D1:], ssum2[:])
            nc.sync.dma_start(out_d[:], esmb[:])

    if do_compile:
        nc.compile()
    return nc


def _make_inmaps(hidden, enc, W):
    """Per-core input dict list: split-precision transposed enc planes."""
    hidden = np.asarray(hidden, dtype=np.float32)
    enc = np.asarray(enc, dtype=np.float32)
    W = np.asarray(W, dtype=np.float32)
    in_maps = []
    for c in range(NCORES):
        b0 = c * BL
        ec = enc[:, b0:b0 + BL, :]                       # [L, BL, H] f32
        hi = ec.astype(np.float16)
        lo = ((ec - hi.astype(np.float32)) * LO_SCALE).astype(
            ml_dtypes.float8_e4m3fn)
        # main groups 0..6: [j, g, hc, b, lg] -> [NJ*NBIG*128, BL*LC]
        def main_layout(x):
            h = x[:(NG - 1) * LC].reshape(NG - 1, LC, BL, NJ, 128)
            h = h.transpose(3, 0, 4, 2, 1)               # j, g, hc, b, lg
            return np.ascontiguousarray(h).reshape(NJ * (NG - 1) * 128, BL * LC)
        # tail group 7: [t, j, hc, b, lgin] -> [TPG*NJ*128, BL*LT]
        def tail_layout(x):
            h = x[(NG - 1) * LC:].reshape(TPG, LT, BL, NJ, 128)
            h = h.transpose(0, 3, 4, 2, 1)               # t, j, hc, b, lgin
            return np.ascontiguousarray(h).reshape(TPG * NJ * 128, BL * LT)
        in_maps.append({
            "hi": main_layout(hi),
            "lo": main_layout(lo),
            "hit": tail_layout(hi),
            "lot": tail_layout(lo),
            "hid": np.ascontiguousarray(hidden[0, b0:b0 + BL, :]),
            "wh": W.astype(np.float16),
            "wl": ((W - W.astype(np.float16).astype(np.float32)) * (2.0 ** 13)
                   ).astype(ml_dtypes.float8_e4m3fn),
        })
    return in_maps


def kernel(hidden, encoder_outputs, W, b):
    if "nc" not in _cache:
        _cache["nc"] = _build()
    nc = _cache["nc"]
    in_maps = _make_inmaps(hidden, encoder_outputs, W)
    res = run_bass_kernel_spmd(nc, in_maps, core_ids=list(range(NCORES)))
    out = np.empty((B, 1, L), dtype=np.float32)
    for c in range(NCORES):
        out[c * BL:(c + 1) * BL, 0, :] = np.asarray(res.results[c]["out"]).astype(np.float32)
    return out
